# revision 30
# baseline (speedup 1.0000x reference)
"""BatchedLensBank Trainium2 kernel.

Computation (per lens n): LayerNorm(x) -> per-lens affine -> 3-layer MLP
  xe[n]    = x_norm * LN_w[n] + LN_b[n]                      [D]
  h1[n]    = relu(W1[n] @ xe[n] + b1[n])                     [H1]
  h2[n]    = relu(W2[n] @ h1[n] + b2[n])                     [H2]
  logits[n]= W3[n,0] @ h2[n] + b3[n,0]                       scalar
  probs    = sigmoid(logits)

Sharding: lens dim N=256 split across 8 cores (32 lenses/core), x replicated.

Per-core mapping (memory-bound on streaming W1, 134 MiB/core):
  j-major partition map p = N_loc*j + n (j in [0,4)), h = T1*j + t.
  64 passes of [128, 4096] W1 tiles; each pass is one fused DVE
  TENSOR_TENSOR_REDUCE (elementwise product + free-dim sum, seeded with b1)
  against a x4-replicated xe tile. DVE streams ~1 elt/cycle/lane (~283 us),
  under the ~375 us HBM floor, so the kernel is DMA-bound as intended.
  All SBUF-side DMA access patterns are contiguous partition blocks.
"""

import numpy as np

M_CORES = 8
J = 4  # replication factor; partitions used = J * N_loc


def _build(N_loc, D, H1, H2, w1_bufs=5, alt_dma=False, fat=1, pe_rep=True):
    from contextlib import ExitStack

    import concourse.bacc as bacc
    import concourse.tile as tile
    from concourse import mybir
    from concourse.dve_ops import TENSOR_TENSOR_REDUCE

    f32 = mybir.dt.float32
    Alu = mybir.AluOpType
    Act = mybir.ActivationFunctionType

    P = J * N_loc  # 128
    T1 = H1 // J  # 64 layer-1 passes
    T2 = H2 // J  # 16 layer-2 passes
    LN_EPS = 1e-5

    def blk(j):
        return slice(N_loc * j, N_loc * (j + 1))

    nc = bacc.Bacc("TRN2", target_bir_lowering=False)

    x_d = nc.dram_tensor("x", [D], f32, kind="ExternalInput")
    lnw_d = nc.dram_tensor("LN_w", [N_loc, D], f32, kind="ExternalInput")
    lnb_d = nc.dram_tensor("LN_b", [N_loc, D], f32, kind="ExternalInput")
    w1_d = nc.dram_tensor("W1", [N_loc, H1, D], f32, kind="ExternalInput")
    b1_d = nc.dram_tensor("b1", [N_loc, H1], f32, kind="ExternalInput")
    w2_d = nc.dram_tensor("W2", [N_loc, H2, H1], f32, kind="ExternalInput")
    b2_d = nc.dram_tensor("b2", [N_loc, H2], f32, kind="ExternalInput")
    w3_d = nc.dram_tensor("W3", [N_loc, 1, H2], f32, kind="ExternalInput")
    b3_d = nc.dram_tensor("b3", [N_loc, 1], f32, kind="ExternalInput")
    probs_d = nc.dram_tensor("probs", [N_loc, 1], f32, kind="ExternalOutput")
    logits_d = nc.dram_tensor("logits", [N_loc, 1], f32, kind="ExternalOutput")

    # DRAM view of W1 with h split as (j, groups of `fat` passes, i):
    # h = T1*j + fat*tf + i ; partition p = N_loc*j + n ; pass t = fat*tf + i
    w1_f = w1_d[:, :, :].rearrange("n (j tf i) d -> j n tf i d", j=J, i=fat)

    with tile.TileContext(nc) as tc, ExitStack() as ctx:
        const = ctx.enter_context(tc.tile_pool(name="const", bufs=1))

        xe = const.tile([P, D], f32)
        b1_t = const.tile([P, T1], f32)
        h1_acc = const.tile([P, T1], f32)
        dummy_p = const.tile([P, 1], f32)

        prep = ctx.enter_context(tc.tile_pool(name="prep", bufs=1))
        eps_t = prep.tile([N_loc, 1], f32)
        nc.vector.memset(eps_t, LN_EPS)
        # warm the Sqrt table set at t~0 so the real sqrt finds it resident
        warm = prep.tile([N_loc, 1], f32)
        nc.scalar.activation(out=warm, in_=eps_t, func=Act.Sqrt)

        if pe_rep:
            # replication matrix R[n, N_loc*j + n'] = (n' == n), via iota:
            # it[n, (j, n')] = n' - n ; R = (it == 0)
            rep_i = prep.tile([N_loc, P], mybir.dt.int32)
            nc.gpsimd.iota(
                rep_i, pattern=[[0, J], [1, N_loc]], base=0, channel_multiplier=-1
            )
            rep = prep.tile([N_loc, P], f32)
            nc.vector.tensor_scalar(
                out=rep, in0=rep_i, scalar1=0, scalar2=None, op0=Alu.is_equal
            )

        xb = prep.tile([N_loc, D], f32)
        nc.gpsimd.dma_start(out=xb, in_=x_d[None, :].broadcast_to((N_loc, D)))
        lnw = prep.tile([N_loc, D], f32)
        nc.sync.dma_start(out=lnw, in_=lnw_d[:, :])
        lnb = prep.tile([N_loc, D], f32)
        nc.sync.dma_start(out=lnb, in_=lnb_d[:, :])

        # layer-1/2/3 biases + small weights, then W2 (2 MiB): emitted after
        # the LN inputs so the xe chain starts immediately; all of it still
        # streams well before it is needed. All per-j block DMAs.
        w2_sb = const.tile([P, T2, H1], f32)
        b2_t = const.tile([P, T2], f32)
        for j in range(J):
            nc.sync.dma_start(out=b1_t[blk(j), :], in_=b1_d[:, T1 * j : T1 * (j + 1)])
        for j in range(J):
            nc.sync.dma_start(
                out=w2_sb[blk(j), :, :], in_=w2_d[:, T2 * j : T2 * (j + 1), :]
            )
            nc.sync.dma_start(out=b2_t[blk(j), :], in_=b2_d[:, T2 * j : T2 * (j + 1)])
        w3_sb = const.tile([N_loc, H2], f32)
        nc.sync.dma_start(out=w3_sb, in_=w3_d[:, 0, :])
        b3_sb = const.tile([N_loc, 1], f32)
        nc.sync.dma_start(out=b3_sb, in_=b3_d[:, :])

        # ---- LayerNorm stats ----
        sub = 512 if D % 512 == 0 else int(np.gcd(512, D))
        nsub = D // sub
        stats = prep.tile([N_loc, nsub, nc.vector.BN_STATS_DIM], f32)
        xb_g = xb.rearrange("p (s f) -> p s f", f=sub)
        for s in range(nsub):
            nc.vector.bn_stats(out=stats[:, s, :], in_=xb_g[:, s, :])
        mv = prep.tile([N_loc, nc.vector.BN_AGGR_DIM], f32)
        nc.vector.bn_aggr(out=mv, in_=stats)

        rstd = prep.tile([N_loc, 1], f32)
        # rstd = 1 / sqrt(var + eps)
        nc.scalar.activation(out=rstd, in_=mv[:, 1:2], func=Act.Sqrt, bias=eps_t)
        nc.vector.reciprocal(out=rstd, in_=rstd)

        # ---- xe_n = (x - mean) * rstd * LN_w + LN_b  at [N_loc, D] ----
        # xb = (xb - mean) * LN_w   (in place)
        nc.vector.scalar_tensor_tensor(
            out=xb, in0=xb, scalar=mv[:, 0:1], in1=lnw,
            op0=Alu.subtract, op1=Alu.mult,
        )
        # lnb = xb * rstd + LN_b    (in place -> xe_n)
        nc.vector.scalar_tensor_tensor(
            out=lnb, in0=xb, scalar=rstd, in1=lnb,
            op0=Alu.mult, op1=Alu.add,
        )

        # ---- replicate xe[N_loc*j + n, :] = xe_n[n, :] ----
        if pe_rep:
            # via PE (bank-by-bank ACT copies pipeline behind the matmuls)
            with tc.tile_pool(name="ps", bufs=1, space="PSUM") as psp:
                xep = psp.tile([P, D], f32)
                nfree = 512
                for c in range(D // nfree):
                    sl = slice(c * nfree, (c + 1) * nfree)
                    nc.tensor.matmul(
                        xep[:, sl], lhsT=rep, rhs=lnb[:, sl], start=True, stop=True
                    )
                    nc.scalar.copy(out=xe[:, sl], in_=xep[:, sl])
        else:
            for j in range(J):
                nc.sync.dma_start(out=xe[blk(j), :], in_=lnb)
        # preload the sigmoid table set now (ACT is idle for the whole W1
        # stream; the ~2.7us table load hides there instead of on the tail)
        nc.scalar.activation(out=warm, in_=eps_t, func=Act.Sigmoid)

        # ---- layer 1: T1 passes over [P, D] W1 tiles ----
        w1p = ctx.enter_context(tc.tile_pool(name="w1p", bufs=w1_bufs))
        for tf in range(T1 // fat):
            wt = w1p.tile([P, fat, D], f32, tag="w1tile")
            eng = nc.scalar if (alt_dma and tf % 2) else nc.sync
            eng.dma_start(out=wt, in_=w1_f[:, :, tf, :, :])
            for i in range(fat):
                t = tf * fat + i
                # h1_acc[:, t] = b1 + sum_d(W1 * xe)
                nc.vector._custom_dve(
                    TENSOR_TENSOR_REDUCE,
                    out=dummy_p.broadcast_to((P, D)),
                    in0=wt[:, i, :], in1=xe,
                    s0=b1_t[:, t : t + 1], s1=1.0,
                    accum_out=h1_acc[:, t : t + 1],
                )

        # relu
        nc.vector.tensor_scalar(
            out=h1_acc, in0=h1_acc, scalar1=0.0, scalar2=None, op0=Alu.max
        )

        # ---- h1_n[n, T1*jj + t] = h1_acc[N_loc*jj + n, t], then x4 rep ----
        h1_n = const.tile([N_loc, H1], f32)
        for jj in range(J):
            nc.sync.dma_start(
                out=h1_n[:, T1 * jj : T1 * (jj + 1)], in_=h1_acc[blk(jj), :]
            )
        h1_rep = const.tile([P, H1], f32)
        for j in range(J):
            nc.sync.dma_start(out=h1_rep[blk(j), :], in_=h1_n)

        # ---- layer 2: T2 passes over [P, H1] W2 tiles ----
        h2_acc = const.tile([P, T2], f32)
        for s in range(T2):
            nc.vector._custom_dve(
                TENSOR_TENSOR_REDUCE,
                out=dummy_p.broadcast_to((P, H1)),
                in0=w2_sb[:, s, :], in1=h1_rep,
                s0=b2_t[:, s : s + 1], s1=1.0,
                accum_out=h2_acc[:, s : s + 1],
            )
        nc.vector.tensor_scalar(
            out=h2_acc, in0=h2_acc, scalar1=0.0, scalar2=None, op0=Alu.max
        )

        # ---- h2_n[n, T2*jj + s] = h2_acc[N_loc*jj + n, s] ----
        h2_n = const.tile([N_loc, H2], f32)
        for jj in range(J):
            nc.sync.dma_start(
                out=h2_n[:, T2 * jj : T2 * (jj + 1)], in_=h2_acc[blk(jj), :]
            )

        # ---- layer 3 + sigmoid ----
        logit = const.tile([N_loc, 1], f32)
        dummy_n = const.tile([N_loc, 1], f32)
        nc.vector._custom_dve(
            TENSOR_TENSOR_REDUCE,
            out=dummy_n.broadcast_to((N_loc, H2)),
            in0=w3_sb, in1=h2_n,
            s0=b3_sb[:, 0:1], s1=1.0,
            accum_out=logit,
        )
        prob = const.tile([N_loc, 1], f32)
        nc.scalar.activation(out=prob, in_=logit, func=Act.Sigmoid)

        nc.sync.dma_start(out=logits_d[:, :], in_=logit)
        nc.sync.dma_start(out=probs_d[:, :], in_=prob)

    nc.compile()
    return nc


_CACHE = {}


def _get_nc(N_loc, D_, H1_, H2_, **kw):
    key = (N_loc, D_, H1_, H2_, tuple(sorted(kw.items())))
    if key not in _CACHE:
        _CACHE[key] = _build(N_loc, D_, H1_, H2_, **kw)
    return _CACHE[key]


def _run(x, LN_w, LN_b, W1, b1, W2, b2, W3, b3, _retries=2, **spmd_kwargs):
    from concourse.bass_utils import run_bass_kernel_spmd

    x = np.ascontiguousarray(np.asarray(x, dtype=np.float32))
    N = LN_w.shape[0]
    N_loc = N // M_CORES
    nc = _get_nc(N_loc, x.shape[0], W1.shape[1], W2.shape[1])

    def shard(a):
        a = np.ascontiguousarray(np.asarray(a, dtype=np.float32))
        return [np.ascontiguousarray(a[c * N_loc : (c + 1) * N_loc]) for c in range(M_CORES)]

    sh = {k: shard(v) for k, v in
          [("LN_w", LN_w), ("LN_b", LN_b), ("W1", W1), ("b1", b1),
           ("W2", W2), ("b2", b2), ("W3", W3), ("b3", b3)]}
    in_maps = [
        {"x": x, **{k: v[c] for k, v in sh.items()}} for c in range(M_CORES)
    ]

    last_exc = None
    for _ in range(_retries + 1):
        try:
            res = run_bass_kernel_spmd(
                nc, in_maps, core_ids=list(range(M_CORES)), **spmd_kwargs
            )
            break
        except Exception as exc:  # transient device faults: reload + retry
            last_exc = exc
            res = None
    if res is None:
        raise last_exc
    probs = np.concatenate([r["probs"][:, 0] for r in res.results])
    logits = np.concatenate([r["logits"][:, 0] for r in res.results])
    return probs.astype(np.float32), logits.astype(np.float32), res


def kernel(x, LN_w, LN_b, W1, b1, W2, b2, W3, b3):
    probs, logits, _ = _run(x, LN_w, LN_b, W1, b1, W2, b2, W3, b3)
    return probs, logits


# revision 33
# speedup vs baseline: 1.0237x; 1.0237x over previous
"""BatchedLensBank Trainium2 kernel.

Computation (per lens n): LayerNorm(x) -> per-lens affine -> 3-layer MLP
  xe[n]    = x_norm * LN_w[n] + LN_b[n]                      [D]
  h1[n]    = relu(W1[n] @ xe[n] + b1[n])                     [H1]
  h2[n]    = relu(W2[n] @ h1[n] + b2[n])                     [H2]
  logits[n]= W3[n,0] @ h2[n] + b3[n,0]                       scalar
  probs    = sigmoid(logits)

Sharding: lens dim N=256 split across 8 cores (32 lenses/core), x replicated.

Per-core mapping (memory-bound on streaming W1, 134 MiB/core):
  j-major partition map p = N_loc*j + n (j in [0,4)), h = T1*j + t.
  64 passes of [128, 4096] W1 tiles; each pass is one fused DVE
  TENSOR_TENSOR_REDUCE (elementwise product + free-dim sum, seeded with b1)
  against a x4-replicated xe tile. DVE streams ~1 elt/cycle/lane (~283 us),
  under the ~375 us HBM floor, so the kernel is DMA-bound as intended.
  All SBUF-side DMA access patterns are contiguous partition blocks.
"""

import numpy as np

M_CORES = 8
J = 4  # replication factor; partitions used = J * N_loc


def _build(N_loc, D, H1, H2, w1_bufs=5, alt_dma=False, fat=1, pe_rep=True):
    from contextlib import ExitStack

    import concourse.bacc as bacc
    import concourse.tile as tile
    from concourse import mybir
    from concourse.dve_ops import TENSOR_TENSOR_REDUCE

    f32 = mybir.dt.float32
    Alu = mybir.AluOpType
    Act = mybir.ActivationFunctionType

    P = J * N_loc  # 128
    T1 = H1 // J  # 64 layer-1 passes
    T2 = H2 // J  # 16 layer-2 passes
    LN_EPS = 1e-5

    def blk(j):
        return slice(N_loc * j, N_loc * (j + 1))

    nc = bacc.Bacc("TRN2", target_bir_lowering=False)

    x_d = nc.dram_tensor("x", [D], f32, kind="ExternalInput")
    lnw_d = nc.dram_tensor("LN_w", [N_loc, D], f32, kind="ExternalInput")
    lnb_d = nc.dram_tensor("LN_b", [N_loc, D], f32, kind="ExternalInput")
    w1_d = nc.dram_tensor("W1", [N_loc, H1, D], f32, kind="ExternalInput")
    b1_d = nc.dram_tensor("b1", [N_loc, H1], f32, kind="ExternalInput")
    w2_d = nc.dram_tensor("W2", [N_loc, H2, H1], f32, kind="ExternalInput")
    b2_d = nc.dram_tensor("b2", [N_loc, H2], f32, kind="ExternalInput")
    w3_d = nc.dram_tensor("W3", [N_loc, 1, H2], f32, kind="ExternalInput")
    b3_d = nc.dram_tensor("b3", [N_loc, 1], f32, kind="ExternalInput")
    probs_d = nc.dram_tensor("probs", [N_loc, 1], f32, kind="ExternalOutput")
    logits_d = nc.dram_tensor("logits", [N_loc, 1], f32, kind="ExternalOutput")

    # DRAM view of W1 with h split as (j, groups of `fat` passes, i):
    # h = T1*j + fat*tf + i ; partition p = N_loc*j + n ; pass t = fat*tf + i
    w1_f = w1_d[:, :, :].rearrange("n (j tf i) d -> j n tf i d", j=J, i=fat)

    with tile.TileContext(nc) as tc, ExitStack() as ctx:
        const = ctx.enter_context(tc.tile_pool(name="const", bufs=1))

        xe = const.tile([P, D], f32)
        b1_t = const.tile([P, T1], f32)
        h1_acc = const.tile([P, T1], f32)
        dummy_p = const.tile([P, 1], f32)

        prep = ctx.enter_context(tc.tile_pool(name="prep", bufs=1))
        eps_t = prep.tile([N_loc, 1], f32)
        nc.vector.memset(eps_t, LN_EPS)
        # warm the Sqrt table set at t~0 so the real sqrt finds it resident
        warm = prep.tile([N_loc, 1], f32)
        nc.scalar.activation(out=warm, in_=eps_t, func=Act.Sqrt)

        if pe_rep:
            # replication matrix R[n, N_loc*j + n'] = (n' == n), via iota:
            # it[n, (j, n')] = n' - n ; R = (it == 0)
            rep_i = prep.tile([N_loc, P], mybir.dt.int32)
            nc.gpsimd.iota(
                rep_i, pattern=[[0, J], [1, N_loc]], base=0, channel_multiplier=-1
            )
            rep = prep.tile([N_loc, P], f32)
            nc.vector.tensor_scalar(
                out=rep, in0=rep_i, scalar1=0, scalar2=None, op0=Alu.is_equal
            )

        # permutation matrices for the inter-layer partition shuffles:
        # perm1[jj][p', p] = (p' == N_loc*jj + p%N_loc)   [P, P]
        # perm2[jj][p', n] = (p' == N_loc*jj + n)         [P, N_loc]
        perm1, perm2 = [], []
        for jj in range(J):
            p1i = prep.tile([P, P], mybir.dt.int32, tag=f"p1i{jj}")
            nc.gpsimd.iota(
                p1i, pattern=[[0, J], [1, N_loc]],
                base=N_loc * jj, channel_multiplier=-1,
            )
            p1 = prep.tile([P, P], f32, tag=f"p1{jj}")
            nc.vector.tensor_scalar(
                out=p1, in0=p1i, scalar1=0, scalar2=None, op0=Alu.is_equal
            )
            perm1.append(p1)
            p2i = prep.tile([P, N_loc], mybir.dt.int32, tag=f"p2i{jj}")
            nc.gpsimd.iota(
                p2i, pattern=[[1, N_loc]],
                base=N_loc * jj, channel_multiplier=-1,
            )
            p2 = prep.tile([P, N_loc], f32, tag=f"p2{jj}")
            nc.vector.tensor_scalar(
                out=p2, in0=p2i, scalar1=0, scalar2=None, op0=Alu.is_equal
            )
            perm2.append(p2)

        xb = prep.tile([N_loc, D], f32)
        nc.gpsimd.dma_start(out=xb, in_=x_d[None, :].broadcast_to((N_loc, D)))
        lnw = prep.tile([N_loc, D], f32)
        nc.sync.dma_start(out=lnw, in_=lnw_d[:, :])
        lnb = prep.tile([N_loc, D], f32)
        nc.sync.dma_start(out=lnb, in_=lnb_d[:, :])

        # layer-1/2/3 biases + small weights, then W2 (2 MiB): emitted after
        # the LN inputs so the xe chain starts immediately; all of it still
        # streams well before it is needed. All per-j block DMAs.
        w2_sb = const.tile([P, T2, H1], f32)
        b2_t = const.tile([P, T2], f32)
        for j in range(J):
            nc.sync.dma_start(out=b1_t[blk(j), :], in_=b1_d[:, T1 * j : T1 * (j + 1)])
        for j in range(J):
            nc.sync.dma_start(
                out=w2_sb[blk(j), :, :], in_=w2_d[:, T2 * j : T2 * (j + 1), :]
            )
            nc.sync.dma_start(out=b2_t[blk(j), :], in_=b2_d[:, T2 * j : T2 * (j + 1)])
        w3_sb = const.tile([N_loc, H2], f32)
        nc.sync.dma_start(out=w3_sb, in_=w3_d[:, 0, :])
        b3_sb = const.tile([N_loc, 1], f32)
        nc.sync.dma_start(out=b3_sb, in_=b3_d[:, :])

        # ---- LayerNorm stats ----
        sub = 512 if D % 512 == 0 else int(np.gcd(512, D))
        nsub = D // sub
        stats = prep.tile([N_loc, nsub, nc.vector.BN_STATS_DIM], f32)
        xb_g = xb.rearrange("p (s f) -> p s f", f=sub)
        for s in range(nsub):
            nc.vector.bn_stats(out=stats[:, s, :], in_=xb_g[:, s, :])
        mv = prep.tile([N_loc, nc.vector.BN_AGGR_DIM], f32)
        nc.vector.bn_aggr(out=mv, in_=stats)

        rstd = prep.tile([N_loc, 1], f32)
        # rstd = 1 / sqrt(var + eps)
        nc.scalar.activation(out=rstd, in_=mv[:, 1:2], func=Act.Sqrt, bias=eps_t)
        nc.vector.reciprocal(out=rstd, in_=rstd)

        # ---- xe_n = (x - mean) * rstd * LN_w + LN_b  at [N_loc, D] ----
        # xb = (xb - mean) * LN_w   (in place)
        nc.vector.scalar_tensor_tensor(
            out=xb, in0=xb, scalar=mv[:, 0:1], in1=lnw,
            op0=Alu.subtract, op1=Alu.mult,
        )
        # lnb = xb * rstd + LN_b    (in place -> xe_n)
        nc.vector.scalar_tensor_tensor(
            out=lnb, in0=xb, scalar=rstd, in1=lnb,
            op0=Alu.mult, op1=Alu.add,
        )

        # ---- replicate xe[N_loc*j + n, :] = xe_n[n, :] ----
        if pe_rep:
            # via PE (bank-by-bank ACT copies pipeline behind the matmuls)
            with tc.tile_pool(name="ps", bufs=1, space="PSUM") as psp:
                xep = psp.tile([P, D], f32)
                nfree = 512
                for c in range(D // nfree):
                    sl = slice(c * nfree, (c + 1) * nfree)
                    nc.tensor.matmul(
                        xep[:, sl], lhsT=rep, rhs=lnb[:, sl], start=True, stop=True
                    )
                    nc.scalar.copy(out=xe[:, sl], in_=xep[:, sl])
        else:
            for j in range(J):
                nc.sync.dma_start(out=xe[blk(j), :], in_=lnb)
        # preload the sigmoid table set now (ACT is idle for the whole W1
        # stream; the ~2.7us table load hides there instead of on the tail)
        nc.scalar.activation(out=warm, in_=eps_t, func=Act.Sigmoid)

        # ---- layer 1: T1 passes over [P, D] W1 tiles ----
        w1p = ctx.enter_context(tc.tile_pool(name="w1p", bufs=w1_bufs))
        for tf in range(T1 // fat):
            wt = w1p.tile([P, fat, D], f32, tag="w1tile")
            eng = nc.scalar if (alt_dma and tf % 2) else nc.sync
            eng.dma_start(out=wt, in_=w1_f[:, :, tf, :, :])
            for i in range(fat):
                t = tf * fat + i
                # h1_acc[:, t] = b1 + sum_d(W1 * xe)
                nc.vector._custom_dve(
                    TENSOR_TENSOR_REDUCE,
                    out=dummy_p.broadcast_to((P, D)),
                    in0=wt[:, i, :], in1=xe,
                    s0=b1_t[:, t : t + 1], s1=1.0,
                    accum_out=h1_acc[:, t : t + 1],
                )

        # ---- h1_rep[p, T1*jj + t] = relu(h1_acc[N_loc*jj + p%N_loc, t]) ----
        # Permutation matmuls on the (idle) PE move h1 between partition
        # layouts; relu rides the ACT PSUM->SBUF copies for free.
        h1_rep = const.tile([P, H1], f32)
        with tc.tile_pool(name="ps2", bufs=J, space="PSUM") as ps2:
            for jj in range(J):
                pst = ps2.tile([P, T1], f32, tag="pst")
                nc.tensor.matmul(
                    pst, lhsT=perm1[jj], rhs=h1_acc, start=True, stop=True
                )
                nc.scalar.activation(
                    out=h1_rep[:, T1 * jj : T1 * (jj + 1)], in_=pst, func=Act.Relu
                )

        # ---- layer 2: T2 passes over [P, H1] W2 tiles ----
        h2_acc = const.tile([P, T2], f32)
        for s in range(T2):
            nc.vector._custom_dve(
                TENSOR_TENSOR_REDUCE,
                out=dummy_p.broadcast_to((P, H1)),
                in0=w2_sb[:, s, :], in1=h1_rep,
                s0=b2_t[:, s : s + 1], s1=1.0,
                accum_out=h2_acc[:, s : s + 1],
            )
        # ---- h2_n[n, T2*jj + s] = relu(h2_acc[N_loc*jj + n, s]) via PE ----
        h2_n = const.tile([N_loc, H2], f32)
        with tc.tile_pool(name="ps3", bufs=J, space="PSUM") as ps3:
            for jj in range(J):
                pst2 = ps3.tile([N_loc, T2], f32, tag="pst2")
                nc.tensor.matmul(
                    pst2, lhsT=perm2[jj], rhs=h2_acc, start=True, stop=True
                )
                nc.scalar.activation(
                    out=h2_n[:, T2 * jj : T2 * (jj + 1)], in_=pst2, func=Act.Relu
                )

        # ---- layer 3 + sigmoid ----
        logit = const.tile([N_loc, 1], f32)
        dummy_n = const.tile([N_loc, 1], f32)
        nc.vector._custom_dve(
            TENSOR_TENSOR_REDUCE,
            out=dummy_n.broadcast_to((N_loc, H2)),
            in0=w3_sb, in1=h2_n,
            s0=b3_sb[:, 0:1], s1=1.0,
            accum_out=logit,
        )
        prob = const.tile([N_loc, 1], f32)
        nc.scalar.activation(out=prob, in_=logit, func=Act.Sigmoid)

        nc.sync.dma_start(out=logits_d[:, :], in_=logit)
        nc.sync.dma_start(out=probs_d[:, :], in_=prob)

    nc.compile()
    return nc


_CACHE = {}


def _get_nc(N_loc, D_, H1_, H2_, **kw):
    key = (N_loc, D_, H1_, H2_, tuple(sorted(kw.items())))
    if key not in _CACHE:
        _CACHE[key] = _build(N_loc, D_, H1_, H2_, **kw)
    return _CACHE[key]


def _run(x, LN_w, LN_b, W1, b1, W2, b2, W3, b3, _retries=2, **spmd_kwargs):
    from concourse.bass_utils import run_bass_kernel_spmd

    x = np.ascontiguousarray(np.asarray(x, dtype=np.float32))
    N = LN_w.shape[0]
    N_loc = N // M_CORES
    nc = _get_nc(N_loc, x.shape[0], W1.shape[1], W2.shape[1])

    def shard(a):
        a = np.ascontiguousarray(np.asarray(a, dtype=np.float32))
        return [np.ascontiguousarray(a[c * N_loc : (c + 1) * N_loc]) for c in range(M_CORES)]

    sh = {k: shard(v) for k, v in
          [("LN_w", LN_w), ("LN_b", LN_b), ("W1", W1), ("b1", b1),
           ("W2", W2), ("b2", b2), ("W3", W3), ("b3", b3)]}
    in_maps = [
        {"x": x, **{k: v[c] for k, v in sh.items()}} for c in range(M_CORES)
    ]

    last_exc = None
    for _ in range(_retries + 1):
        try:
            res = run_bass_kernel_spmd(
                nc, in_maps, core_ids=list(range(M_CORES)), **spmd_kwargs
            )
            break
        except Exception as exc:  # transient device faults: reload + retry
            last_exc = exc
            res = None
    if res is None:
        raise last_exc
    probs = np.concatenate([r["probs"][:, 0] for r in res.results])
    logits = np.concatenate([r["logits"][:, 0] for r in res.results])
    return probs.astype(np.float32), logits.astype(np.float32), res


def kernel(x, LN_w, LN_b, W1, b1, W2, b2, W3, b3):
    probs, logits, _ = _run(x, LN_w, LN_b, W1, b1, W2, b2, W3, b3)
    return probs, logits
